# revision 11
# baseline (speedup 1.0000x reference)
"""Multi-head self-attention Trainium2 Bass kernel.

Sharding: tensor-parallel over heads — 16 heads / 8 cores = 2 heads per core.
Each core computes q,k,v for its 2 heads from the full token stream, runs
attention, and produces a partial output projection (its 128 inner dims of
w_out); the host sums the 8 partials and adds the bias.

Per-core layouts (tokens t = b*2048 + n, 8192 total):
  xT    [1024, 8192]  x transposed (host prep), fp32->fp32r
  wqkv  [1024, 384]   per-core [q(128) | k(128) | v(128)] columns
  wout  [128, 1024]   per-core rows of w_out
  out   [8192, 1024]  partial output (host sums over cores)

On-chip per batch b:
  qT,kT [128, 2048]   (2 heads x 64 dims on partitions; fp32r)
  v     [128, 16, 128] natural layout tiles (PE-transposed from vT; bf16)
  S^T   [j 128, i 512] score tiles via row-packed K=64 matmul pairs (fp32r)
  P^T   exp(S^T/8) via ACT psum->sbuf, bf16
  A^T   [128, 512] via col-packed M=64 bf16 matmul pairs, heads at rows 0-63/64-127
  sums  via 4-wide col-packed ones-matmuls (M=1 at psum rows 0/32/64/96)
  out   A^T-slice.T @ wout -> natural [tok 128, feat 512] tiles (fp32r)
"""
import numpy as np

import concourse.bass as bass
import concourse.mybir as mybir
import concourse.tile as tile
from concourse import bacc
from concourse.masks import make_identity

dt = mybir.dt
F32, F32R, BF16 = dt.float32, dt.float32r, dt.bfloat16
AF = mybir.ActivationFunctionType

B, SEQ, D = 4, 2048, 1024
NCORES = 8
TOK = B * SEQ              # 8192
NCH = SEQ // 512           # 4 token chunks per batch
NJT = SEQ // 128           # 16 key tiles per batch
SCALE = 64 ** -0.5         # 0.125


def build_nc(nreps: int = 1):
    nc = bacc.Bacc(trn_type="TRN2")
    xT = nc.dram_tensor("xT", [D, TOK], F32R, kind="ExternalInput")
    wqkv = nc.dram_tensor("wqkv", [D, 384], F32R, kind="ExternalInput")
    wout = nc.dram_tensor("wout", [128, D], F32R, kind="ExternalInput")
    out = nc.dram_tensor("out", [TOK, D], F32, kind="ExternalOutput")

    xT3 = xT.rearrange("(ko p) t -> p ko t", p=128)       # [128, 8, 8192]
    wq3 = wqkv.rearrange("(ko p) f -> p ko f", p=128)     # [128, 8, 384]

    with tile.TileContext(nc) as tc:
        with tc.tile_pool(name="const", bufs=1) as constp, \
             tc.tile_pool(name="sb", bufs=1) as sb, \
             tc.tile_pool(name="ps", bufs=1, space="PSUM") as ps:

            # --- constants / weights (loaded once) ---
            w_sb = constp.tile([128, 8, 384], F32R, tag="w_sb")
            nc.sync.dma_start(out=w_sb, in_=wq3[:, :, :])
            wout_sb = constp.tile([128, D], F32R, tag="wout_sb")
            nc.sync.dma_start(out=wout_sb, in_=wout[:, :])
            ident_bf = constp.tile([128, 128], BF16, tag="ident_bf")
            make_identity(nc, ident_bf[:, :])
            ones_bf = constp.tile([128, 1], BF16, tag="ones_bf")
            nc.vector.memset(ones_bf[:, :], 1.0)

            def body():
                for b in range(B):
                    t0 = b * SEQ
                    # ---------------- Phase A: qT/kT/vT + v transpose ------
                    qT = sb.tile([128, SEQ], F32R, tag="qT", bufs=2, name=f"qT{b}")
                    kT = sb.tile([128, SEQ], F32R, tag="kT", bufs=2, name=f"kT{b}")
                    v_sb = sb.tile([128, NJT, 128], BF16, tag="v_sb", bufs=2, name=f"v{b}")
                    for ch in range(NCH):
                        c0 = t0 + ch * 512
                        xts = []
                        for ko in range(8):
                            xt = sb.tile([128, 512], F32R, tag="xt", bufs=16,
                                         name=f"xt{b}_{ch}_{ko}")
                            nc.sync.dma_start(out=xt, in_=xT3[:, ko, c0:c0 + 512])
                            xts.append(xt)
                        for dst, col0 in ((qT, 0), (kT, 128)):
                            pp = ps.tile([128, 512], F32, tag="misc", bufs=2,
                                         name=f"pj{b}_{ch}_{col0}")
                            for ko in range(8):
                                nc.tensor.matmul(
                                    pp[:, :], w_sb[:, ko, col0:col0 + 128],
                                    xts[ko][:, :],
                                    start=(ko == 0), stop=(ko == 7))
                            nc.vector.tensor_copy(
                                dst[:, ch * 512:(ch + 1) * 512], pp[:, :])
                        pv = ps.tile([128, 512], F32, tag="misc", bufs=2,
                                     name=f"pv{b}_{ch}")
                        for ko in range(8):
                            nc.tensor.matmul(
                                pv[:, :], w_sb[:, ko, 256:384], xts[ko][:, :],
                                start=(ko == 0), stop=(ko == 7))
                        vstage = sb.tile([128, 512], BF16, tag="vstage", bufs=2,
                                         name=f"vs{b}_{ch}")
                        nc.vector.tensor_copy(vstage[:, :], pv[:, :])
                        for t in range(4):
                            vt_ps = ps.tile([128, 512], BF16, tag="misc",
                                            bufs=2, name=f"vt{b}_{ch}_{t}")
                            nc.tensor.transpose(
                                vt_ps[:, 0:128],
                                vstage[:, t * 128:(t + 1) * 128],
                                ident_bf[:, :])
                            nc.vector.tensor_copy(
                                v_sb[:, ch * 4 + t, :], vt_ps[:, 0:128])

                    # ---------------- Phase B: attention -------------------
                    for ic in range(NCH):
                        i0 = ic * 512
                        pT = [sb.tile([128, NJT, 512], BF16, tag="pT", bufs=4,
                                      name=f"pT{b}_{ic}_{h}") for h in range(2)]
                        for g in range(NJT // 2):
                            for h in range(2):
                                st = ps.tile([128, 1024], F32, tag="st", bufs=2,
                                             name=f"st{b}_{ic}_{g}_{h}")
                                for jj in range(2):
                                    jt = 2 * g + jj
                                    nc.tensor.matmul(
                                        st[:, jj * 512:(jj + 1) * 512],
                                        kT[h * 64:(h + 1) * 64,
                                           jt * 128:(jt + 1) * 128],
                                        qT[h * 64:(h + 1) * 64, i0:i0 + 512],
                                        start=True, stop=True,
                                        tile_position=(h * 64, 0))
                                nc.scalar.activation(
                                    pT[h][:, 2 * g:2 * g + 2, :], st[:, :],
                                    AF.Exp, bias=0.0, scale=SCALE)
                        at = ps.tile([128, 512], F32, tag="at", bufs=2,
                                     name=f"at{b}_{ic}")
                        sums = ps.tile([128, 512], F32, tag="at", bufs=2,
                                       name=f"sums{b}_{ic}")
                        for jt in range(NJT):
                            nc.tensor.matmul(
                                at[0:64, :], v_sb[:, jt, 0:64], pT[0][:, jt, :],
                                start=(jt == 0), stop=(jt == NJT - 1),
                                tile_position=(0, 0))
                            nc.tensor.matmul(
                                at[64:128, :], v_sb[:, jt, 64:128],
                                pT[1][:, jt, :],
                                start=(jt == 0), stop=(jt == NJT - 1),
                                tile_position=(0, 64))
                        for jt in range(NJT):
                            for h in range(2):
                                r0 = h * 64
                                nc.tensor.matmul(
                                    sums[r0:r0 + 1, :], ones_bf[:, 0:1],
                                    pT[h][:, jt, :],
                                    start=(jt == 0), stop=(jt == NJT - 1),
                                    tile_position=(0, r0))
                        rs = sb.tile([128, 512], F32R, tag="rs", bufs=2,
                                     name=f"rs{b}_{ic}")
                        for h in range(2):
                            with nc.allow_low_precision("softmax denom f32r"):
                                nc.vector.reciprocal(
                                    rs[h * 64:h * 64 + 1, :],
                                    sums[h * 64:h * 64 + 1, :])
                        rsl = sb.tile([1, 512], F32R, tag="rsl", bufs=2,
                                      name=f"rsl{b}_{ic}")
                        nc.sync.dma_start(out=rsl[0:1, :], in_=rs[64:65, :])
                        rbs0 = sb.tile([128, 512], F32R, tag="rbs0", bufs=2,
                                       name=f"rbs0{b}_{ic}")
                        rbs1 = sb.tile([128, 512], F32R, tag="rbs1", bufs=2,
                                       name=f"rbs1{b}_{ic}")
                        nc.gpsimd.partition_broadcast(
                            rbs0[:, :], rs[0:1, :], channels=128)
                        nc.gpsimd.partition_broadcast(
                            rbs1[:, :], rsl[0:1, :], channels=128)
                        atn = sb.tile([128, 512], F32R, tag="atn", bufs=4,
                                      name=f"atn{b}_{ic}")
                        nc.vector.tensor_mul(atn[0:64, :], at[0:64, :],
                                             rbs0[0:64, :])
                        nc.vector.tensor_mul(atn[64:128, :], at[64:128, :],
                                             rbs1[64:128, :])

                        # ------------- Phase C: output projection ----------
                        for tt in range(4):
                            for nf in range(2):
                                op = ps.tile([128, 512], F32, tag="misc",
                                             bufs=2,
                                             name=f"op{b}_{ic}_{tt}_{nf}")
                                nc.tensor.matmul(
                                    op[:, :],
                                    atn[:, tt * 128:(tt + 1) * 128],
                                    wout_sb[:, nf * 512:(nf + 1) * 512],
                                    start=True, stop=True)
                                o_sb = sb.tile([128, 512], F32, tag="o_sb", bufs=4,
                                               name=f"o{b}_{ic}_{tt}_{nf}")
                                nc.vector.tensor_copy(o_sb[:, :], op[:, :])
                                r = t0 + i0 + tt * 128
                                nc.sync.dma_start(
                                    out=out[r:r + 128, nf * 512:(nf + 1) * 512],
                                    in_=o_sb[:, :])

            if nreps == 1:
                body()
            else:
                with tc.For_i(0, nreps, 1):
                    body()
    nc.finalize()
    return nc


def _shard_inputs(x, w_qkv, w_out):
    xT = np.ascontiguousarray(x.reshape(TOK, D).T).astype(np.float32)
    in_maps = []
    for c in range(NCORES):
        wq = w_qkv[:, c * 128:(c + 1) * 128]
        wk = w_qkv[:, D + c * 128:D + (c + 1) * 128]
        wv = w_qkv[:, 2 * D + c * 128:2 * D + (c + 1) * 128]
        wqkv_c = np.ascontiguousarray(
            np.concatenate([wq, wk, wv], axis=1)).astype(np.float32)
        wout_c = np.ascontiguousarray(
            w_out[c * 128:(c + 1) * 128, :]).astype(np.float32)
        in_maps.append({"xT": xT, "wqkv": wqkv_c, "wout": wout_c})
    return in_maps


_CACHE = {}


def kernel(x, w_qkv, w_out, b_out):
    x = np.asarray(x, dtype=np.float32)
    w_qkv = np.asarray(w_qkv, dtype=np.float32)
    w_out = np.asarray(w_out, dtype=np.float32)
    b_out = np.asarray(b_out, dtype=np.float32)

    from concourse.bass_utils import run_bass_kernel_spmd
    if "nc" not in _CACHE:
        _CACHE["nc"] = build_nc()
    nc = _CACHE["nc"]
    in_maps = _shard_inputs(x, w_qkv, w_out)
    res = run_bass_kernel_spmd(nc, in_maps, core_ids=list(range(NCORES)))
    acc = np.zeros((TOK, D), dtype=np.float64)
    for c in range(NCORES):
        acc += res.results[c]["out"].astype(np.float64)
    result = (acc + b_out.astype(np.float64)).astype(np.float32)
    return result.reshape(B, SEQ, D)
